# revision 16
# baseline (speedup 1.0000x reference)
"""JunctionGNN Trainium2 kernel: 3-layer GCN + edge MLP over 100k nodes / 1.6M edges.

Sharding: edges sorted by destination (col); core k owns a contiguous slice of
nslice dst nodes and its incoming edges. Per 128-node dst block, edges are
grouped by source-row table chunk (4 chunks, int16-indexable) and padded to
uniform per-chunk tile counts so one NEFF serves all 8 cores. Blocks are
processed in super-blocks of SB so each dma_gather covers SB blocks (amortizes
the ~1us SWDGE fixed cost per gather call).

Layer 0 is computed replicated: every core computes g0 = relu(x@Wn+b)@Wc0*dinv
for ALL nodes directly into its local DRAM table (no AllGather). Layers 1-2
AllGather a [NG,128] bf16 table (g in cols 0:64). Aggregation uses a bf16
one-hot built by is_equal(iota, colrel) and bf16 PE matmuls into PSUM; the
epilogue (self-loop, dinv, bias, LayerNorm, ReLU, residual) is batched per
super-block, then fused with the next layer's g = h@Wc (PE transpose + matmul)
or, after layer 2, with the u-table build ([u_a|u_b] = h@W1a|h@W1b, bf16).

Edge MLP runs feature-major in bf16: e = relu(eaT@We), er accumulated in PSUM
from W1c@e plus per-edge gathered u_a[row]/u_b[col] folded in with identity
matmuls, then two more matmuls for the head projections.
"""

import numpy as np

import concourse.bass as bass
import concourse.bacc as bacc
import concourse.mybir as mybir
import concourse.tile as tile
from concourse import bass_utils
from concourse import library_config

P = 128
H = 64
EPS = 1e-5
NCORES = 8
NCHUNK = 4


def _ceil(a, b):
    return (a + b - 1) // b


def _pick_sb(nblk):
    for sb in (7, 8, 6, 5, 4):
        if nblk % sb == 0:
            return sb
    return 1


def _prep(inputs):
    """Host-side index preprocessing + per-core input construction."""
    x = np.asarray(inputs["x"], np.float32)
    edge_attr = np.asarray(inputs["edge_attr"], np.float32)
    ei = np.asarray(inputs["edge_index"]).astype(np.int64)
    N = x.shape[0]
    E = ei.shape[1]
    EA = edge_attr.shape[1]

    row, col = ei[0], ei[1]
    perm = np.argsort(col, kind="stable")
    row_s = row[perm]
    col_s = col[perm]

    nslice = _ceil(_ceil(N, NCORES), P) * P          # nodes per core slice
    nblk = nslice // P                               # dst blocks per core
    NG = nslice * NCORES                             # gathered table rows
    CH = NG // NCHUNK                                # table chunk rows (int16-safe)
    assert CH % P == 0 and CH <= 32768
    SB = _pick_sb(nblk)
    nsb = nblk // SB

    deg = np.zeros(NG, np.float32)
    deg[:N] = np.bincount(col, minlength=N).astype(np.float32)
    deg += 1.0
    dinv = (1.0 / np.sqrt(deg)).astype(np.float32)

    nblk_g = nblk * NCORES
    block_lo = np.searchsorted(col_s, np.arange(0, NG, P))
    block_hi = np.searchsorted(col_s, np.arange(P, NG + P, P))
    rchunk_s = row_s // CH

    # uniform per-chunk tile counts Tc over all (block, chunk)
    cnt = np.zeros((nblk_g, NCHUNK), np.int64)
    for gb in range(nblk_g):
        lo, hi = int(block_lo[gb]), int(block_hi[gb])
        if hi > lo:
            cnt[gb] = np.bincount(rchunk_s[lo:hi], minlength=NCHUNK)
    Tc = [int(_ceil(int(cnt[:, c].max()), P)) for c in range(NCHUNK)]
    Tc = [max(t, 1) for t in Tc]
    TT = sum(Tc)
    C0 = np.cumsum([0] + Tc)                          # per-block tile offsets
    C0s = np.cumsum([0] + [SB * t for t in Tc])       # per-super-block offsets
    Ec = nblk * TT * P
    TcA = np.asarray(Tc)

    in_maps, meta = [], []
    for k in range(NCORES):
        # gather indices in (super-block, chunk, block-within) order
        idx16 = np.zeros((nsb, 16, SB * TT * 8), np.int16)
        cidx16 = np.zeros((nsb, 16, SB * TT * 8), np.int16)
        colrel = np.full((nblk, P, TT), -1.0, np.float32)
        eaT = np.zeros((Ec, EA), np.float32)
        flat_pos, sort_idx = [], []
        for b in range(nblk):
            gb = k * nblk + b
            sbi, j = b // SB, b % SB
            lo, hi = int(block_lo[gb]), int(block_hi[gb])
            n = hi - lo
            if n == 0:
                continue
            rc = rchunk_s[lo:hi]
            order = np.argsort(rc, kind="stable")
            rows = row_s[lo:hi][order]
            cols = col_s[lo:hi][order]
            ccnt = np.bincount(rc, minlength=NCHUNK)
            # position within the block's padded chunk segment
            j_in_chunk = np.arange(n) - np.repeat(
                np.cumsum(np.concatenate([[0], ccnt[:-1]])), ccnt
            )
            jpos = C0[rc[order]] * P + j_in_chunk            # block-local slot
            # position within the super-block gather call of chunk c
            i_call = (C0s[rc[order]] + j * TcA[rc[order]]) * P + j_in_chunk
            idx16[sbi, i_call % 16, i_call // 16] = (rows - rc[order] * CH).astype(
                np.int16
            )
            cidx16[sbi, i_call % 16, i_call // 16] = (cols - k * nslice).astype(
                np.int16
            )
            colrel[b, jpos % P, jpos // P] = (cols - gb * P).astype(np.float32)
            base = b * TT * P
            eaT[base + jpos] = edge_attr[perm[lo:hi][order]]
            flat_pos.append(base + jpos)
            sort_idx.append(np.arange(lo, hi)[order])
        meta.append(
            (
                np.concatenate(flat_pos) if flat_pos else np.zeros(0, np.int64),
                np.concatenate(sort_idx) if sort_idx else np.zeros(0, np.int64),
            )
        )

        xT = np.zeros((2, nslice), np.float32)
        s0, s1 = k * nslice, min((k + 1) * nslice, N)
        if s1 > s0:
            xT[:, : s1 - s0] = x[s0:s1].T
        dinvs = dinv[k * nslice : (k + 1) * nslice].reshape(nblk, P).T.copy()

        in_maps.append(
            {
                "idx16": np.tile(idx16, (1, 8, 1)).reshape(nsb * 128, SB * TT * 8),
                "cidx16": np.tile(cidx16, (1, 8, 1)).reshape(nsb * 128, SB * TT * 8),
                "colrel": colrel.reshape(nblk * P, TT).astype(np.float16),
                "eaT": np.ascontiguousarray(eaT.T).astype(np.float16),
                "xT": xT.astype(np.float16),
                "dinv": dinvs,
            }
        )

    xTf = np.zeros((2, NG), np.float32)
    xTf[:, :N] = x.T
    dinvf = dinv.reshape(nblk_g, P).T.copy()

    W_ep1 = np.asarray(inputs["W_ep1"], np.float32)
    bf = lambda a: np.asarray(a, np.float32).astype(np.float16)
    shared = {
        "xTf": xTf.astype(np.float16),
        "dinvf": dinvf,
        "W_node": bf(inputs["W_node"]),
        "b_node_c": np.asarray(inputs["b_node"], np.float32).reshape(H, 1),
        "b_node_bc": np.broadcast_to(
            np.asarray(inputs["b_node"], np.float32), (P, H)
        ).copy(),
        "W_edge": bf(inputs["W_edge"]),
        "b_edge_c": np.asarray(inputs["b_edge"], np.float32).reshape(H, 1),
        "W1a": bf(W_ep1[0:H]),
        "W1b": bf(W_ep1[H : 2 * H]),
        "W1c": bf(W_ep1[2 * H : 3 * H]),
        "b_ep1_c": np.asarray(inputs["b_ep1"], np.float32).reshape(H, 1),
        "W_ep2": bf(inputs["W_ep2"]),
        "b_ep2_c": np.asarray(inputs["b_ep2"], np.float32).reshape(H // 2, 1),
        "W_heads": bf(
            np.concatenate(
                [
                    np.asarray(inputs["W_from"], np.float32),
                    np.asarray(inputs["W_to"], np.float32),
                    np.asarray(inputs["W_turn"], np.float32),
                ],
                axis=1,
            )
        ),
        "b_heads_c": np.array(
            [inputs["b_from"][0], inputs["b_to"][0], inputs["b_turn"][0]], np.float32
        ).reshape(3, 1),
        "id_bf16": np.eye(P, dtype=np.float16),
        "iota_bf": np.tile(np.arange(P, dtype=np.float32), (P, 1)).astype(np.float16),
    }
    for i in range(3):
        shared[f"Wc{i}"] = bf(np.asarray(inputs["W_conv"], np.float32)[i])
        shared[f"bc{i}"] = np.broadcast_to(
            np.asarray(inputs["b_conv"], np.float32)[i], (P, H)
        ).copy()
        shared[f"lg{i}"] = np.broadcast_to(
            np.asarray(inputs["ln_g"], np.float32)[i], (P, H)
        ).copy()
        shared[f"lb{i}"] = np.broadcast_to(
            np.asarray(inputs["ln_b"], np.float32)[i], (P, H)
        ).copy()
    for m in in_maps:
        m.update(shared)

    dims = dict(N=N, E=E, EA=EA, nslice=nslice, nblk=nblk, NG=NG, CH=CH,
                Tc=Tc, TT=TT, Ec=Ec, SB=SB, nsb=nsb)
    return dims, in_maps, meta, perm


def _build(dims):
    f32 = mybir.dt.float32
    f32r = mybir.dt.float32r
    bf16 = mybir.dt.float16
    i16 = mybir.dt.int16
    AX = mybir.AxisListType
    OP = mybir.AluOpType
    AF = mybir.ActivationFunctionType

    nslice, nblk, NG, CH, Tc, TT, Ec, EA, SB, nsb = (
        dims["nslice"], dims["nblk"], dims["NG"], dims["CH"],
        dims["Tc"], dims["TT"], dims["Ec"], dims["EA"], dims["SB"], dims["nsb"],
    )
    nblk_g = nblk * NCORES
    C0 = np.cumsum([0] + list(Tc))
    C0s = np.cumsum([0] + [SB * t for t in Tc])
    rg = [list(range(NCORES))]

    nc = bacc.Bacc(
        "TRN2", target_bir_lowering=False, debug=False,
        enable_asserts=False, num_devices=NCORES,
    )

    d_idx16 = nc.dram_tensor("idx16", [nsb * 128, SB * TT * 8], i16, kind="ExternalInput").ap()
    d_cidx16 = nc.dram_tensor("cidx16", [nsb * 128, SB * TT * 8], i16, kind="ExternalInput").ap()
    d_colrel = nc.dram_tensor("colrel", [nblk * P, TT], bf16, kind="ExternalInput").ap()
    d_eaT = nc.dram_tensor("eaT", [EA, Ec], bf16, kind="ExternalInput").ap()
    d_xT = nc.dram_tensor("xT", [2, nslice], bf16, kind="ExternalInput").ap()
    d_xTf = nc.dram_tensor("xTf", [2, NG], bf16, kind="ExternalInput").ap()
    d_dinv = nc.dram_tensor("dinv", [P, nblk], f32, kind="ExternalInput").ap()
    d_dinvf = nc.dram_tensor("dinvf", [P, nblk_g], f32, kind="ExternalInput").ap()

    din = {}
    consts = [
        ("W_node", [2, H], bf16), ("b_node_c", [H, 1], f32),
        ("b_node_bc", [P, H], f32),
        ("W_edge", [EA, H], bf16), ("b_edge_c", [H, 1], f32),
        ("W1a", [H, H], bf16), ("W1b", [H, H], bf16), ("W1c", [H, H], bf16),
        ("b_ep1_c", [H, 1], f32), ("W_ep2", [H, H // 2], bf16),
        ("b_ep2_c", [H // 2, 1], f32), ("W_heads", [H // 2, 3], bf16),
        ("b_heads_c", [3, 1], f32),
        ("id_bf16", [P, P], bf16), ("iota_bf", [P, P], bf16),
    ]
    for i in range(3):
        consts += [(f"Wc{i}", [H, H], bf16), (f"bc{i}", [P, H], f32),
                   (f"lg{i}", [P, H], f32), (f"lb{i}", [P, H], f32)]
    for name, shp, dt in consts:
        din[name] = nc.dram_tensor(name, shp, dt, kind="ExternalInput").ap()

    d_out = nc.dram_tensor("out", [3, Ec], f32, kind="ExternalOutput").ap()

    g_tab = nc.dram_tensor("g_tab", [NG, 2 * H], bf16, kind="Internal").ap()
    ag_g_in = nc.dram_tensor("ag_g_in", [nslice, 2 * H], bf16, kind="Internal").ap()
    g_full = nc.dram_tensor("g_full", [NG, 2 * H], bf16, kind="Internal", addr_space="Shared").ap()
    ag_u_in = nc.dram_tensor("ag_u_in", [nslice, 2 * H], bf16, kind="Internal").ap()
    u_full = nc.dram_tensor("u_full", [NG, 2 * H], bf16, kind="Internal", addr_space="Shared").ap()

    with tile.TileContext(nc) as tc:
        nc.gpsimd.load_library(library_config.mlp)
        cp = tc.alloc_tile_pool(name="consts", bufs=1)
        sb = {}
        for name, shp, dt in consts:
            t = cp.tile(shp, dt, name=f"sb_{name}")
            nc.sync.dma_start(out=t[:], in_=din[name])
            sb[name] = t

        # GCN-phase persistent buffers, released before the edge MLP
        cp2 = tc.alloc_tile_pool(name="gcnwork", bufs=1)
        dinv_sb = cp2.tile([P, nblk], f32, name="dinv_sb")
        nc.sync.dma_start(out=dinv_sb[:], in_=d_dinv)
        dinvf_sb = cp2.tile([P, nblk_g], f32, name="dinvf_sb")
        nc.sync.dma_start(out=dinvf_sb[:], in_=d_dinvf)
        colrel_sb = cp2.tile([P, nblk, TT], bf16, name="colrel_sb")
        nc.sync.dma_start(
            out=colrel_sb[:],
            in_=d_colrel.rearrange("(b p) t -> p b t", p=P),
        )
        hbuf = cp2.tile([P, nblk * H], f32, name="hbuf")
        gbuf = cp2.tile([P, nblk * H], f32, name="gbuf")

        # ---- layer-0 table: g0 for ALL nodes, computed replicated
        L0B = 8  # blocks per table-write batch
        with (
            tc.tile_pool(name="l0ps", bufs=3, space="PSUM") as l0p,
            tc.tile_pool(name="l0sb", bufs=3) as l0s,
        ):
            for gq in range(nblk_g // L0B):
                xb = l0s.tile([2, L0B * P], bf16, name="xb")
                nc.sync.dma_start(
                    out=xb[:], in_=d_xTf[:, gq * L0B * P : (gq + 1) * L0B * P]
                )
                gt8 = l0s.tile([P, L0B, H], bf16, name="gt8")
                g0_ps = l0p.tile([P, L0B, H], f32, name="g0_ps")
                for ji in range(L0B):
                    gb = gq * L0B + ji
                    h0T_ps = l0p.tile([H, P], f32, name="h0T_ps")
                    nc.tensor.matmul(
                        out=h0T_ps[:], lhsT=sb["W_node"][:],
                        rhs=xb[:, ji * P : (ji + 1) * P], start=True, stop=True,
                    )
                    h0T = l0s.tile([H, P], bf16, name="h0T")
                    nc.scalar.activation(
                        out=h0T[:], in_=h0T_ps[:], func=AF.Relu,
                        bias=sb["b_node_c"][:, 0:1],
                    )
                    nc.tensor.matmul(
                        out=g0_ps[:, ji, :], lhsT=h0T[:], rhs=sb["Wc0"][:],
                        start=True, stop=True, skip_group_check=True,
                    )
                nc.vector.tensor_tensor(
                    out=gt8[:],
                    in0=g0_ps[:],
                    in1=dinvf_sb[:, gq * L0B : (gq + 1) * L0B].rearrange(
                        "p (b o) -> p b o", o=1
                    ).to_broadcast([P, L0B, H]),
                    op=OP.mult,
                )
                nc.sync.dma_start(
                    out=g_tab[gq * L0B * P : (gq + 1) * L0B * P, 0:H].rearrange(
                        "(j p) h -> p j h", p=P
                    ),
                    in_=gt8[:],
                )

        # ---- own h0 (f32, node-major) + own g0 into gbuf
        with (
            tc.tile_pool(name="h0ps", bufs=2, space="PSUM") as hp,
            tc.tile_pool(name="h0sb", bufs=3) as hs,
        ):
            for gq in range(nblk // L0B + (1 if nblk % L0B else 0)):
                nb = min(L0B, nblk - gq * L0B)
                xb = hs.tile([2, L0B * P], bf16, name="oxb")
                nc.sync.dma_start(
                    out=xb[:, : nb * P],
                    in_=d_xT[:, gq * L0B * P : gq * L0B * P + nb * P],
                )
                for ji in range(nb):
                    b = gq * L0B + ji
                    ps = hp.tile([P, H], f32, name="h0ps")
                    nc.tensor.matmul(
                        out=ps[:], lhsT=xb[:, ji * P : (ji + 1) * P],
                        rhs=sb["W_node"][:], start=True, stop=True,
                    )
                    hb = hbuf[:, b * H : (b + 1) * H]
                    nc.vector.tensor_tensor(out=hb, in0=ps[:], in1=sb["b_node_bc"][:], op=OP.add)
                    nc.vector.tensor_scalar_max(hb, hb, 0.0)
                    hbb = hs.tile([P, H], bf16, name="hbb")
                    nc.any.tensor_copy(out=hbb[:], in_=hb)
                    hT_ps = hp.tile([H, P], bf16, name="hT_ps")
                    nc.tensor.transpose(out=hT_ps[:], in_=hbb[:], identity=sb["id_bf16"][:])
                    hT = hs.tile([H, P], bf16, name="hT")
                    nc.any.tensor_copy(out=hT[:], in_=hT_ps[:])
                    g_ps = hp.tile([P, H], f32, name="g_ps")
                    nc.tensor.matmul(
                        out=g_ps[:], lhsT=hT[:], rhs=sb["Wc0"][:], start=True, stop=True
                    )
                    nc.vector.tensor_scalar(
                        out=gbuf[:, b * H : (b + 1) * H], in0=g_ps[:],
                        scalar1=dinv_sb[:, b : b + 1], scalar2=None, op0=OP.mult,
                    )

        # ---- 3 GCN layers
        for li in range(3):
            table = g_tab if li == 0 else g_full
            with (
                tc.tile_pool(name=f"sweep{li}", bufs=2) as sp,
                tc.tile_pool(name=f"swsmall{li}", bufs=3) as ss,
                tc.tile_pool(name=f"sweep_ps{li}", bufs=2, space="PSUM") as spp,
                tc.tile_pool(name=f"prep_ps{li}", bufs=2, space="PSUM") as ppp,
                tc.tile_pool(name=f"epi{li}", bufs=2) as ep,
            ):
                for sbi in range(nsb):
                    idxs = ss.tile([128, SB * TT * 8], i16, name="idxs")
                    nc.sync.dma_start(
                        out=idxs[:], in_=d_idx16[sbi * 128 : (sbi + 1) * 128, :]
                    )
                    gt = sp.tile([P, SB * TT, 2 * H], bf16, name="gt")
                    for c in range(NCHUNK):
                        nc.gpsimd.dma_gather(
                            out_ap=gt[:, C0s[c] : C0s[c + 1], :],
                            in_ap=table[c * CH : (c + 1) * CH, :],
                            idxs_ap=idxs[:, C0s[c] * 8 : C0s[c + 1] * 8],
                            num_idxs=SB * Tc[c] * P,
                            num_idxs_reg=SB * Tc[c] * P,
                            elem_size=2 * H,
                            single_packet=False,
                        )
                    agg = spp.tile([P, SB, H], f32, name="agg")
                    for j in range(SB):
                        b = sbi * SB + j
                        oh = ss.tile([P, TT, P], bf16, name="oh")
                        nc.vector.tensor_tensor(
                            out=oh[:],
                            in0=sb["iota_bf"][:].rearrange(
                                "p (o f) -> p o f", o=1
                            ).to_broadcast([P, TT, P]),
                            in1=colrel_sb[:, b, :].rearrange(
                                "p (t o) -> p t o", o=1
                            ).to_broadcast([P, TT, P]),
                            op=OP.is_equal,
                        )
                        mi = 0
                        for c in range(NCHUNK):
                            for t in range(Tc[c]):
                                nc.tensor.matmul(
                                    out=agg[:, j, :],
                                    lhsT=oh[:, C0[c] + t, :],
                                    rhs=gt[:, C0s[c] + j * Tc[c] + t, 0:H],
                                    start=(mi == 0), stop=(mi == TT - 1),
                                    skip_group_check=True,
                                )
                                mi += 1

                    # ---- batched epilogue over SB blocks
                    cols = slice(sbi * SB * H, (sbi + 1) * SB * H)
                    dj = dinv_sb[:, sbi * SB : (sbi + 1) * SB].rearrange(
                        "p (b o) -> p b o", o=1
                    ).to_broadcast([P, SB, H])
                    hc = hbuf[:, cols].rearrange("p (b h) -> p b h", h=H)
                    gc = gbuf[:, cols].rearrange("p (b h) -> p b h", h=H)
                    pre = ep.tile([P, SB, H], f32, name="pre")
                    nc.vector.tensor_tensor(out=pre[:], in0=agg[:], in1=gc, op=OP.add)
                    nc.vector.tensor_tensor(out=pre[:], in0=pre[:], in1=dj, op=OP.mult)
                    nc.vector.tensor_tensor(
                        out=pre[:], in0=pre[:],
                        in1=sb[f"bc{li}"][:].rearrange("p (o h) -> p o h", o=1).to_broadcast([P, SB, H]),
                        op=OP.add,
                    )
                    mu = ep.tile([P, SB, 1], f32, name="mu")
                    nc.vector.tensor_reduce(out=mu[:], in_=pre[:], axis=AX.X, op=OP.add)
                    nc.vector.tensor_scalar_mul(mu[:], mu[:], 1.0 / H)
                    xc = ep.tile([P, SB, H], f32, name="xc")
                    nc.vector.tensor_tensor(
                        out=xc[:], in0=pre[:], in1=mu[:].to_broadcast([P, SB, H]),
                        op=OP.subtract,
                    )
                    sq = ep.tile([P, SB, H], f32, name="sq")
                    vs = ep.tile([P, SB, 1], f32, name="vs")
                    nc.scalar.activation(
                        out=sq[:].rearrange("p b h -> p (b h)"),
                        in_=xc[:].rearrange("p b h -> p (b h)"),
                        func=AF.Square,
                    )
                    nc.vector.tensor_reduce(out=vs[:], in_=sq[:], axis=AX.X, op=OP.add)
                    nc.vector.tensor_scalar(
                        out=vs[:], in0=vs[:], scalar1=1.0 / H, scalar2=EPS,
                        op0=OP.mult, op1=OP.add,
                    )
                    nc.vector.reciprocal(out=vs[:], in_=vs[:])
                    nc.scalar.sqrt(out=vs[:], in_=vs[:])
                    nc.vector.tensor_tensor(
                        out=xc[:], in0=xc[:], in1=vs[:].to_broadcast([P, SB, H]),
                        op=OP.mult,
                    )
                    nc.vector.tensor_tensor(
                        out=xc[:], in0=xc[:],
                        in1=sb[f"lg{li}"][:].rearrange("p (o h) -> p o h", o=1).to_broadcast([P, SB, H]),
                        op=OP.mult,
                    )
                    nc.vector.tensor_tensor(
                        out=xc[:], in0=xc[:],
                        in1=sb[f"lb{li}"][:].rearrange("p (o h) -> p o h", o=1).to_broadcast([P, SB, H]),
                        op=OP.add,
                    )
                    nc.vector.tensor_scalar_max(xc[:], xc[:], 0.0)
                    nc.vector.tensor_tensor(out=hc, in0=xc[:], in1=hc, op=OP.add)

                    # ---- fused next-layer prep (g for li<2, u for li==2)
                    ag_in = ag_g_in if li < 2 else ag_u_in
                    hbb = ep.tile([P, SB, H], bf16, name="hbb")
                    nc.any.tensor_copy(
                        out=hbb[:].rearrange("p b h -> p (b h)"), in_=hbuf[:, cols]
                    )
                    stg = ep.tile([P, SB, 2 * H], bf16, name="stg")
                    for j in range(SB):
                        b = sbi * SB + j
                        hT_ps = ppp.tile([H, P], bf16, name="phT_ps")
                        nc.tensor.transpose(
                            out=hT_ps[:], in_=hbb[:, j, :], identity=sb["id_bf16"][:]
                        )
                        hT = ep.tile([H, P], bf16, name="phT")
                        nc.any.tensor_copy(out=hT[:], in_=hT_ps[:])
                        if li < 2:
                            g_ps = ppp.tile([P, H], f32, name="pg_ps")
                            nc.tensor.matmul(
                                out=g_ps[:], lhsT=hT[:], rhs=sb[f"Wc{li + 1}"][:],
                                start=True, stop=True,
                            )
                            nc.vector.tensor_scalar(
                                out=gbuf[:, b * H : (b + 1) * H], in0=g_ps[:],
                                scalar1=dinv_sb[:, b : b + 1], scalar2=None,
                                op0=OP.mult,
                            )
                            nc.any.tensor_copy(
                                out=stg[:, j, 0:H],
                                in_=gbuf[:, b * H : (b + 1) * H],
                            )
                        else:
                            for jj, wname in enumerate(("W1a", "W1b")):
                                u_ps = ppp.tile([P, H], f32, name="pu_ps")
                                nc.tensor.matmul(
                                    out=u_ps[:], lhsT=hT[:], rhs=sb[wname][:],
                                    start=True, stop=True,
                                )
                                nc.any.tensor_copy(
                                    out=stg[:, j, jj * H : (jj + 1) * H],
                                    in_=u_ps[:],
                                )
                    rows = slice(sbi * SB * P, (sbi + 1) * SB * P)
                    if li < 2:
                        nc.sync.dma_start(
                            out=ag_in[rows, 0:H].rearrange("(b p) h -> p b h", p=P),
                            in_=stg[:, :, 0:H],
                        )
                    else:
                        nc.sync.dma_start(
                            out=ag_in[rows, :].rearrange("(b p) h -> p b h", p=P),
                            in_=stg[:],
                        )

                ag_in = ag_g_in if li < 2 else ag_u_in
                nc.gpsimd.collective_compute(
                    "AllGather", OP.bypass, replica_groups=rg,
                    ins=[ag_in], outs=[g_full if li < 2 else u_full],
                )
        cp2.release()

        # ---- edge MLP (feature-major, bf16)
        tgroups = []
        t0 = 0
        while t0 < TT:
            tgroups.append((t0, min(4, TT - t0)))
            t0 += 4
        with (
            tc.tile_pool(name="fsweep", bufs=2) as fp,
            tc.tile_pool(name="fsmall", bufs=2) as fs,
            tc.tile_pool(name="fsweep_ps", bufs=2, space="PSUM") as fpp,
        ):
            for sbi in range(nsb):
                idxs = fs.tile([128, SB * TT * 8], i16, name="fidxs")
                nc.sync.dma_start(
                    out=idxs[:], in_=d_idx16[sbi * 128 : (sbi + 1) * 128, :]
                )
                cidx = fs.tile([128, SB * TT * 8], i16, name="fcidx")
                nc.sync.dma_start(
                    out=cidx[:], in_=d_cidx16[sbi * 128 : (sbi + 1) * 128, :]
                )
                uar = fp.tile([P, SB * TT, 2 * H], bf16, name="uar")
                ubr = fp.tile([P, SB * TT, 2 * H], bf16, name="ubr")
                for c in range(NCHUNK):
                    nc.gpsimd.dma_gather(
                        out_ap=uar[:, C0s[c] : C0s[c + 1], :],
                        in_ap=u_full[c * CH : (c + 1) * CH, :],
                        idxs_ap=idxs[:, C0s[c] * 8 : C0s[c + 1] * 8],
                        num_idxs=SB * Tc[c] * P,
                        num_idxs_reg=SB * Tc[c] * P,
                        elem_size=2 * H,
                        single_packet=False,
                    )
                    nc.gpsimd.dma_gather(
                        out_ap=ubr[:, C0s[c] : C0s[c + 1], :],
                        in_ap=ag_u_in[:],
                        idxs_ap=cidx[:, C0s[c] * 8 : C0s[c + 1] * 8],
                        num_idxs=SB * Tc[c] * P,
                        num_idxs_reg=SB * Tc[c] * P,
                        elem_size=2 * H,
                        single_packet=False,
                    )
                for j in range(SB):
                    b = sbi * SB + j
                    ea = fs.tile([EA, TT * P], bf16, name="ea")
                    nc.sync.dma_start(
                        out=ea[:], in_=d_eaT[:, b * TT * P : (b + 1) * TT * P]
                    )
                    outw = fs.tile([3, TT * P], f32, name="outw")
                    for c0g, csz in tgroups:
                        Ech = csz * P
                        gcols = slice(c0g * P, c0g * P + Ech)
                        e_ps = fpp.tile([H, 4 * P], f32, name="e_ps")
                        nc.tensor.matmul(
                            out=e_ps[:, :Ech], lhsT=sb["W_edge"][:], rhs=ea[:, gcols],
                            start=True, stop=True,
                        )
                        eT = fs.tile([H, 4 * P], bf16, name="eT")
                        nc.scalar.activation(
                            out=eT[:, :Ech], in_=e_ps[:, :Ech], func=AF.Relu,
                            bias=sb["b_edge_c"][:, 0:1],
                        )
                        er_ps = fpp.tile([H, 4 * P], f32, name="er_ps")
                        nc.tensor.matmul(
                            out=er_ps[:, :Ech], lhsT=sb["W1c"][:],
                            rhs=eT[:, :Ech],
                            start=True, stop=False,
                            skip_group_check=True,
                        )
                        for tt in range(csz):
                            tau = c0g + tt
                            c = int(np.searchsorted(C0, tau, side="right")) - 1
                            st = C0s[c] + j * Tc[c] + (tau - C0[c])
                            nc.tensor.matmul(
                                out=er_ps[:, tt * P : (tt + 1) * P],
                                lhsT=uar[:, st, 0:H],
                                rhs=sb["id_bf16"][:], start=False, stop=False,
                                skip_group_check=True,
                            )
                            nc.tensor.matmul(
                                out=er_ps[:, tt * P : (tt + 1) * P],
                                lhsT=ubr[:, st, H : 2 * H],
                                rhs=sb["id_bf16"][:], start=False, stop=True,
                                skip_group_check=True,
                            )
                        erT = fs.tile([H, 4 * P], bf16, name="erT")
                        nc.scalar.activation(
                            out=erT[:, :Ech], in_=er_ps[:, :Ech], func=AF.Relu,
                            bias=sb["b_ep1_c"][:, 0:1],
                        )
                        er2_ps = fpp.tile([H // 2, 4 * P], f32, name="er2_ps")
                        nc.tensor.matmul(
                            out=er2_ps[:, :Ech], lhsT=sb["W_ep2"][:],
                            rhs=erT[:, :Ech],
                            start=True, stop=True,
                        )
                        er2 = fs.tile([H // 2, 4 * P], bf16, name="er2")
                        nc.scalar.activation(
                            out=er2[:, :Ech], in_=er2_ps[:, :Ech], func=AF.Relu,
                            bias=sb["b_ep2_c"][:, 0:1],
                        )
                        s_ps = fpp.tile([3, 4 * P], f32, name="s_ps")
                        nc.tensor.matmul(
                            out=s_ps[:, :Ech], lhsT=sb["W_heads"][:],
                            rhs=er2[:, :Ech],
                            start=True, stop=True,
                        )
                        nc.vector.tensor_scalar(
                            out=outw[:, gcols], in0=s_ps[:, :Ech],
                            scalar1=sb["b_heads_c"][:, 0:1], scalar2=None, op0=OP.add,
                        )
                    nc.sync.dma_start(
                        out=d_out[:, b * TT * P : (b + 1) * TT * P], in_=outw[:]
                    )
        cp.release()
    nc.compile()
    return nc


def kernel(**inputs):
    dims, in_maps, meta, perm = _prep(inputs)
    nc = _build(dims)
    res = bass_utils.run_bass_kernel_spmd(nc, in_maps, core_ids=list(range(NCORES)))
    E = dims["E"]
    s_sorted = np.zeros((3, E), np.float32)
    for k in range(NCORES):
        flat_pos, sort_idx = meta[k]
        if len(sort_idx):
            s_sorted[:, sort_idx] = res.results[k]["out"][:, flat_pos]
    s = np.zeros((3, E), np.float32)
    s[:, perm] = s_sorted
    return s[0], s[1], s[2]
